# revision 34
# baseline (speedup 1.0000x reference)
"""MatchBRNN Trainium2 kernel: 2-layer action-conditioned-attention +
bidirectional SRU, data-parallel over batch on 8 NeuronCores (B=16 ->
2 batches/core).

Layout C: on-chip column index for (position q, batch b) is
    C(q, b) = (q // 128) * 256 + b * 128 + (q % 128)
i.e. 128-position chunks, batch-major inside a chunk.

Per-core structure (four (layer, chunk) "stages", software-pipelined):
  xtT[(b,k), l] = (x_b @ w1a_b)^T          (block-diag over b; layer-invariant)
  per stage: ytT = (out @ w2a)^T + bias; per 16-s block: tp = xt + yt_s on
  DVE (a few lead slices fused into ACT via tanh-bias to hide stage-start
  latency), one big ACT tanh per block, per-s PE score matmuls vs block-diag
  va; the softmax/pools/SRU tail of stage k-1 is emitted interleaved into
  stage k's blocks so neither ACT nor DVE stalls at chunk boundaries; the
  final stage's tail is split into two q-granules so most of it hides behind
  the last blocks.

All matmul operands staged in bf16 (memT/memr/w1/w2/wsru/eT/pools/h0);
gates/scan in f32; output DMA'd as bf16.  Measured on trn2: ~167 us device
makespan per core (NTFF), rel err ~5e-3 vs the fp32 jax reference.
"""
import numpy as np
import concourse.bass as bass
import concourse.mybir as mybir
import concourse.tile as tile
from concourse.bass_utils import run_bass_kernel_spmd

AF = mybir.ActivationFunctionType
OP = mybir.AluOpType
F32 = mybir.dt.float32
BF16 = mybir.dt.bfloat16
BF16_NP = mybir.dt.np(BF16)

B, S, D = 16, 256, 256
H, NL, A, K = 128, 2, 8, 64
NCORES = 8
B2 = B // NCORES
# fused (ACT-bias) tanh slices: only the first block of each stage
# fuses a few slices so ACT can start before the DVE tp-build catches up
FUSED_FIRST = 10
# final-tail granules: (q-offset, width); last one is the exposed tail
GRAN_BOUNDS = ((0, 64), (64, 64))


def _split_excess_waits(nc, max_waits=1):
    """walrus in this toolchain rejects >1 sem-wait per instruction; hoist
    extras onto same-engine NoOps inserted just before the instruction."""
    n = 0
    for f in nc.m.functions:
        for bb in f.blocks:
            out = []
            for inst in bb.instructions:
                si = inst.sync_info
                waits = list(si.on_wait) if si is not None and si.on_wait else []
                if len(waits) > max_waits:
                    keep, extra = waits[-max_waits:], waits[:-max_waits]
                    for w in extra:
                        n += 1
                        out.append(mybir.InstNoOp(
                            name=f"{inst.name}_ws{n}", engine=inst.engine,
                            ins=[], outs=[],
                            sync_info=mybir.SyncInfo(on_wait=[w], on_update=[])))
                    inst.sync_info = mybir.SyncInfo(
                        on_wait=keep, on_update=list(si.on_update or []))
                out.append(inst)
            bb.instructions = out
    return n


def _build(apply_mask: bool):
    nc = bass.Bass("TRN2")
    dram = nc.dram_tensor
    memT_d = dram("memT", [128, 1024], BF16, kind="ExternalInput")
    memr_d = dram("memr", [128, 1024], BF16, kind="ExternalInput")
    w1_d = dram("w1blk", [128, 512], BF16, kind="ExternalInput")
    w2_d = dram("w2blk", [128, 512], BF16, kind="ExternalInput")
    va_d = dram("vablk", [128, 2], BF16, kind="ExternalInput")
    yb_d = dram("ybias", [128, 1], F32, kind="ExternalInput")
    ws_d = dram("wsru", [128, 8192], BF16, kind="ExternalInput")
    bs_d = dram("bsru", [128, 8], F32, kind="ExternalInput")
    oc_d = dram("onescol", [128, 1], BF16, kind="ExternalInput")
    or_d = dram("onesrow", [1, 128], F32, kind="ExternalInput")
    if apply_mask:
        mk_d = dram("maskmul", [128, 4], F32, kind="ExternalInput")
    outT_d = dram("outT", [2, 128, 512], BF16, kind="ExternalOutput")

    with tile.TileContext(nc) as tc:
        with (
            nc.allow_low_precision(reason="bf16 staging is intentional"),
            tc.tile_pool(name="const", bufs=1) as cp,
            tc.tile_pool(name="work", bufs=1) as wp,
            tc.tile_pool(name="blk", bufs=3) as bp,
            tc.tile_pool(name="ps", bufs=1, space="PSUM") as ps,
        ):
            # ACT table preload: tiny tanh right at t=0, concurrent with DMAs
            warm = cp.tile([128, 1], F32, tag="warm")
            nc.vector.memset(warm[:], 0.0)
            nc.scalar.activation(warm[:], warm[:], AF.Tanh)

            memT = cp.tile([128, 1024], BF16, tag="memT")
            memr = cp.tile([128, 1024], BF16, tag="memr")
            w1 = cp.tile([128, 512], BF16, tag="w1")
            w2 = cp.tile([128, 512], BF16, tag="w2")
            va = cp.tile([128, 2], BF16, tag="va")
            yb = cp.tile([128, 1], F32, tag="yb")
            wsru = cp.tile([128, 8192], BF16, tag="wsru")
            bsru = cp.tile([128, 8], F32, tag="bsru")
            onc = cp.tile([128, 1], BF16, tag="onc")
            onr = cp.tile([1, 128], F32, tag="onr")
            # priority loads first (feed the first chunk)
            for q in (0, 2, 1, 3):
                nc.sync.dma_start(memT[:, q * 256:(q + 1) * 256],
                                  memT_d[:, q * 256:(q + 1) * 256])
            for t, d in ((w1, w1_d), (w2, w2_d), (va, va_d), (yb, yb_d),
                         (onc, oc_d), (onr, or_d), (bsru, bs_d)):
                nc.sync.dma_start(t[:], d[:])
            nc.sync.dma_start(memr[:], memr_d[:])
            for q in range(4):
                nc.sync.dma_start(wsru[:, q * 2048:(q + 1) * 2048],
                                  ws_d[:, q * 2048:(q + 1) * 2048])
            if apply_mask:
                mk = cp.tile([128, 4], F32, tag="mk")
                nc.sync.dma_start(mk[:], mk_d[:])

            h0 = [wp.tile([128, 512], BF16, tag=f"h0{d}", name=f"h0{d}")
                  for d in range(2)]
            h1 = [wp.tile([128, 512], BF16, tag=f"h1{d}", name=f"h1{d}")
                  for d in range(2)]

            # PSUM: 8 banks, all as (128, 512) f32 tiles
            u_ps = {}
            for jj in range(4):
                u_ps[jj] = ps.tile([128, 512], F32, tag=f"u{jj}", name=f"ups{jj}")
            sc_ps = [ps.tile([128, 512], F32, tag=f"sc{h}", name=f"scps{h}")
                     for h in range(2)]
            pn_ps = [ps.tile([128, 512], F32, tag=f"pn{dh}", name=f"pnps{dh}")
                     for dh in range(2)]

            # ---- layer-invariant xtT: contract (b, d-half), block-diag w1 ----
            xt16 = wp.tile([128, 256], BF16, tag="xt16")

            def emit_xtT(ck):
                co = ck * 256
                for cc in range(4):
                    b, ci = cc // 2, cc % 2
                    nc.tensor.matmul(
                        sc_ps[0][:, co:co + 128], w1[:, cc * 128:(cc + 1) * 128],
                        memT[:, ci * 512 + co + b * 128:
                             ci * 512 + co + (b + 1) * 128],
                        start=(cc == 0), stop=(cc == 3))
                nc.vector.tensor_copy(xt16[:, ck * 128:(ck + 1) * 128],
                                      sc_ps[0][:, co:co + 128])

            emit_xtT(0)

            # per-layer tiles (tag-shared across layers; tails never overlap)
            yts = [wp.tile([128, 256], F32, tag=f"yt{li}", name=f"yt{li}")
                   for li in range(NL)]
            eTs = [wp.tile([128, 1024], BF16, tag=f"eT{li}", name=f"eT{li}")
                   for li in range(NL)]
            rzs = [wp.tile([1, 512], F32, tag=f"rz{li}", name=f"rz{li}")
                   for li in range(NL)]
            rzbs = [wp.tile([128, 512], F32, tag=f"rzb{li}", name=f"rzb{li}")
                    for li in range(NL)]
            poolsTs = [[wp.tile([128, 512], BF16, tag=f"pT{li}{dh}",
                                name=f"poolsT{li}_{dh}") for dh in range(2)]
                       for li in range(NL)]
            gts = [[{nm: wp.tile([128, 512], F32, tag=f"{nm}{li}{dr}",
                                 name=f"{nm}_{li}_{dr}")
                     for nm in ("tf", "f", "g", "bin", "c", "tc2", "tr",
                                "dd", "rd2")} for dr in range(2)]
                   for li in range(NL)]

            def emit_ytT(li, ck):
                co = ck * 256
                for cc in range(4):
                    b, ci = cc // 2, cc % 2
                    if li == 0:
                        rhs = memT[:, ci * 512 + co + b * 128:
                                   ci * 512 + co + (b + 1) * 128]
                    else:
                        rhs = h0[ci][:, co + b * 128: co + (b + 1) * 128]
                    nc.tensor.matmul(
                        sc_ps[1][:, co:co + 128],
                        w2[:, cc * 128:(cc + 1) * 128], rhs,
                        start=(cc == 0), stop=(cc == 3))
                nc.vector.tensor_scalar(
                    yts[li][:, ck * 128:(ck + 1) * 128],
                    sc_ps[1][:, co:co + 128], yb[:], None, OP.add)

            def emit_tp(li, ck, blk, nf):
                """DVE tp-build for slices j >= nf of one 16-s block."""
                yt = yts[li]
                tp = bp.tile([128, 4096], BF16, tag="tpre")
                for j in range(nf, 16):
                    s = ck * 128 + blk * 16 + j
                    nc.vector.tensor_scalar(
                        tp[:, (j - nf) * 256:(j - nf + 1) * 256], xt16[:],
                        yt[:, s:s + 1], None, OP.add)
                return tp

            def emit_block(li, ck, blk, nf=0, tp=None):
                """tanh + scores for one 16-s block (tp may be prebuilt)."""
                co = ck * 256
                yt = yts[li]
                tb = bp.tile([128, 4096], BF16, tag="tblk")
                if tp is None:
                    tp = emit_tp(li, ck, blk, nf)
                # fused ACT-bias tanh for j < nf (no DVE work)
                for j in range(nf):
                    s = ck * 128 + blk * 16 + j
                    nc.scalar.activation(tb[:, j * 256:(j + 1) * 256],
                                         xt16[:], AF.Tanh,
                                         bias=yt[:, s:s + 1])
                nw = (16 - nf) * 256
                nc.scalar.activation(tb[:, nf * 256: nf * 256 + nw],
                                     tp[:, 0:nw], AF.Tanh)
                for j in range(16):
                    q = blk * 16 + j
                    for h in range(2):
                        nc.tensor.matmul(
                            sc_ps[h][:, co + q: co + q + 129: 128],
                            tb[:, j * 256 + h * 128: j * 256 + (h + 1) * 128],
                            va[:], start=True, stop=True)

            def emit_tail_piece(li, ck, piece, g=None):
                """Tail ops for one chunk; g=None does the full 128-q chunk,
                g in (0,1) does the half-granule q in [g*64, g*64+64)."""
                co = ck * 256
                eT, rz, rzb = eTs[li], rzs[li], rzbs[li]
                poolsT = poolsTs[li]
                if g is None:
                    spans = [(co, 256)]           # contiguous, both batches
                    bspans = [(co, 128), (co + 128, 128)]
                else:
                    g0, gw = GRAN_BOUNDS[g]
                    spans = [(co + g0, gw), (co + 128 + g0, gw)]
                    bspans = spans
                if piece == 0:
                    for h in range(2):
                        for st, ln in spans:
                            nc.scalar.activation(
                                eT[:, h * 512 + st: h * 512 + st + ln],
                                sc_ps[h][:, st:st + ln], AF.Exp)
                    if apply_mask:
                        for h in range(2):
                            for st, ln in bspans:
                                b = (st - co) // 128
                                sl = eT[:, h * 512 + st: h * 512 + st + ln]
                                nc.vector.tensor_scalar(
                                    sl, sl, mk[:, h * 2 + b: h * 2 + b + 1],
                                    None, OP.mult)
                elif piece == 1:
                    for st, ln in spans:
                        for h in range(2):
                            nc.tensor.matmul(
                                pn_ps[0][0:1, st:st + ln], onc[:],
                                eT[:, h * 512 + st: h * 512 + st + ln],
                                start=(h == 0), stop=(h == 1))
                        nc.vector.reciprocal(rz[0:1, st:st + ln],
                                             pn_ps[0][0:1, st:st + ln])
                elif piece == 2:
                    for st, ln in bspans:
                        nc.tensor.matmul(
                            pn_ps[1][:, st:st + ln], onr[:],
                            rz[0:1, st:st + ln], start=True, stop=True)
                    for st, ln in spans:
                        nc.vector.tensor_copy(rzb[:, st:st + ln],
                                              pn_ps[1][:, st:st + ln])
                elif piece == 3:
                    for dh in range(2):
                        for st, ln in bspans:
                            b = (st - co) // 128
                            for lh in range(2):
                                nc.tensor.matmul(
                                    pn_ps[dh][:, st:st + ln],
                                    memr[:, lh * 512 + b * 256 + dh * 128:
                                         lh * 512 + b * 256 + (dh + 1) * 128],
                                    eT[:, lh * 512 + st: lh * 512 + st + ln],
                                    start=(lh == 0), stop=(lh == 1))
                        for st, ln in spans:
                            nc.vector.scalar_tensor_tensor(
                                poolsT[dh][:, st:st + ln],
                                pn_ps[dh][:, st:st + ln],
                                1.0, rzb[:, st:st + ln], OP.mult, OP.mult)
                elif piece in (4, 6):
                    dr = 0 if piece == 4 else 1
                    for st, ln in spans:
                        for c in range(4):
                            if c < 2:
                                rhs = (memT[:, c * 512 + st: c * 512 + st + ln]
                                       if li == 0 else h0[c][:, st:st + ln])
                            else:
                                rhs = poolsT[c - 2][:, st:st + ln]
                            for jj in range(4):
                                w_off = (((li * 2 + dr) * 16) + c * 4 + jj) * 128
                                nc.tensor.matmul(
                                    u_ps[jj][:, st:st + ln],
                                    wsru[:, w_off:w_off + 128], rhs,
                                    start=(c == 0), stop=(c == 3))
                elif piece in (5, 7):
                    dr = 0 if piece == 5 else 1
                    bcol = (li * 2 + dr) * 2
                    gt = gts[li][dr]
                    tf_, f_, g_, bin_, c_, tc2, tr_, dd_, rd2_ = (
                        gt["tf"], gt["f"], gt["g"], gt["bin"], gt["c"],
                        gt["tc2"], gt["tr"], gt["dd"], gt["rd2"])
                    for st, ln in spans:
                        nc.scalar.activation(tf_[:, st:st + ln],
                                             u_ps[1][:, st:st + ln], AF.Tanh,
                                             bias=bsru[:, bcol:bcol + 1],
                                             scale=0.5)
                        nc.vector.tensor_scalar(f_[:, st:st + ln],
                                                tf_[:, st:st + ln], 0.5, 0.5,
                                                OP.mult, OP.add)
                        nc.vector.tensor_scalar(g_[:, st:st + ln],
                                                tf_[:, st:st + ln], -0.5, 0.5,
                                                OP.mult, OP.add)
                        nc.vector.tensor_tensor(bin_[:, st:st + ln],
                                                g_[:, st:st + ln],
                                                u_ps[0][:, st:st + ln], OP.mult)
                    for st, ln in bspans:
                        qoff = (st - co) % 128
                        nsc = ln
                        if qoff == 0 and ck == 0:
                            init = 0.0
                        elif qoff == 0:
                            init = c_[:, st - 129: st - 128]
                        else:
                            init = c_[:, st - 1: st]
                        nc.vector.tensor_tensor_scan(
                            c_[:, st:st + nsc], f_[:, st:st + nsc],
                            bin_[:, st:st + nsc], init, OP.mult, OP.add)
                    for st, ln in spans:
                        nc.scalar.activation(tc2[:, st:st + ln],
                                             c_[:, st:st + ln], AF.Tanh)
                        nc.scalar.activation(tr_[:, st:st + ln],
                                             u_ps[2][:, st:st + ln], AF.Tanh,
                                             bias=bsru[:, bcol + 1:bcol + 2],
                                             scale=0.5)
                        nc.vector.tensor_tensor(dd_[:, st:st + ln],
                                                tc2[:, st:st + ln],
                                                u_ps[3][:, st:st + ln],
                                                OP.subtract)
                        nc.vector.scalar_tensor_tensor(
                            rd2_[:, st:st + ln], tr_[:, st:st + ln], 1.0,
                            dd_[:, st:st + ln], OP.add, OP.mult)
                        h_t = h0[dr] if li == 0 else h1[dr]
                        nc.vector.scalar_tensor_tensor(
                            h_t[:, st:st + ln], rd2_[:, st:st + ln], 0.5,
                            u_ps[3][:, st:st + ln], OP.mult, OP.add)
                        if li == 1 and piece == 7:
                            for dh in range(2):
                                nc.sync.dma_start(outT_d[dh, :, st:st + ln],
                                                  h1[dh][:, st:st + ln])

            # ---- pipelined emission over 4 stages (li, ck) ----
            stages = [(0, 0), (0, 1), (1, 0), (1, 1)]
            # tail pieces of stage k-1 interleaved into stage k's blocks;
            # ytT of stage k+1 hoisted after block YTT_AT so the PE reaches
            # it before the last blocks' score matmuls
            TAIL_SCHED = {0: [0, 1], 1: [2, 3], 2: [4], 3: [5], 4: [6], 5: [7]}
            YTT_AT = 6
            # final-stage tail: split into NGRAN q-granules; granule g needs
            # scores only from blocks < 2(g+1), so all but the last granule
            # hide behind the remaining blocks
            # ordering constraints: piece p of granule g must follow piece p
            # of the stage-2 tail (TAIL_SCHED: p5 at blk3, p7 at blk5) and of
            # granule g-1 (scan chaining), and granule g's piece 0 needs the
            # scores of blocks < 2*(g+1)
            G_SCHED = {4: [(0, 0), (0, 1)],
                       5: [(0, 2), (0, 3)],
                       6: [(0, 4), (0, 5)],
                       7: [(0, 6), (0, 7)]}
            G_END = [(1, p) for p in range(8)]
            emit_ytT(0, 0)
            emit_xtT(1)
            for k, (li, ck) in enumerate(stages):
                for blk in range(8):
                    emit_block(li, ck, blk,
                               nf=FUSED_FIRST if blk == 0 else 0)
                    if k >= 1:
                        pli, pck = stages[k - 1]
                        for piece in TAIL_SCHED.get(blk, []):
                            emit_tail_piece(pli, pck, piece)
                    if k == 3:
                        for gg, piece in G_SCHED.get(blk, []):
                            emit_tail_piece(1, 1, piece, g=gg)
                    if blk == YTT_AT and k + 1 < 4:
                        emit_ytT(*stages[k + 1])
            for gg, piece in G_END:
                emit_tail_piece(1, 1, piece, g=gg)

    _split_excess_waits(nc)
    return nc


_CACHE = {}


def _get_nc(apply_mask: bool):
    if apply_mask not in _CACHE:
        _CACHE[apply_mask] = _build(apply_mask)
    return _CACHE[apply_mask]


def _c_layout_unused(arr_pos_b):
    """(pos, b, ...) -> columns in layout C: [ck*256 + b*128 + q]."""
    P2, Bb = arr_pos_b.shape[0], arr_pos_b.shape[1]
    rest = arr_pos_b.shape[2:]
    a = arr_pos_b.reshape(2, 128, Bb, *rest)       # (ck, q, b, ...)
    a = np.moveaxis(a, 2, 1)                       # (ck, b, q, ...)
    return a.reshape(512, *rest)


def make_in_maps(x, x_mask, actions, w1, b1, w2, b2, v,
                 sru_w_f, sru_b_f, sru_w_b, sru_b_b):
    x = np.asarray(x, np.float32)
    x_mask = np.asarray(x_mask)
    actions = np.asarray(actions).astype(np.int64)
    w1 = np.asarray(w1, np.float32); b1 = np.asarray(b1, np.float32)
    w2 = np.asarray(w2, np.float32); b2 = np.asarray(b2, np.float32)
    v = np.asarray(v, np.float32)

    apply_mask = bool(x_mask.any())

    # wsru[:, ((li*2+dr)*16 + c*4 + jj)*128 + m] = sru_w[dr][li, c*128+dp, jj*128+m]
    sw = np.stack([np.asarray(sru_w_f, np.float32),
                   np.asarray(sru_w_b, np.float32)], 1)   # (li, dr, 512, 512)
    blk = sw.reshape(NL, 2, 4, 128, 4, 128)               # li dr c dp jj m
    wsru = np.ascontiguousarray(
        blk.transpose(3, 0, 1, 2, 4, 5).reshape(128, 8192)).astype(BF16_NP)
    sb = np.stack([np.asarray(sru_b_f, np.float32),
                   np.asarray(sru_b_b, np.float32)], 1)   # (li, dr, 256)
    bsru = np.ascontiguousarray(
        (0.5 * sb.reshape(NL, 2, 2, 128)).transpose(3, 0, 1, 2).reshape(128, 8))

    # layout C over all cores at once
    xs = x.reshape(NCORES, B2, S, D)
    arr = xs.transpose(0, 2, 1, 3)                         # (core, l, b, d)
    colsC = (arr.reshape(NCORES, 2, 128, B2, D)
             .transpose(0, 1, 3, 2, 4).reshape(NCORES, 512, D))
    # memT[dp, dh*512 + C] = colsC[C, dh*128+dp]
    tmp = colsC.reshape(NCORES, 512, 2, 128)               # (core, C, dh, dp)
    memT_all = np.ascontiguousarray(
        tmp.transpose(0, 3, 2, 1)                          # (core, dp, dh, C)
        .reshape(NCORES, 128, 1024)).astype(BF16_NP)
    # memr[lp, lh*512 + b*256 + d] = x[b, lh*128+lp, d]
    memr_all = np.ascontiguousarray(
        arr.reshape(NCORES, 2, 128, B2 * D).transpose(0, 2, 1, 3)
        .reshape(NCORES, 128, 1024)).astype(BF16_NP)

    a_all = actions.reshape(NCORES, B2)
    in_maps = []
    onescol = np.ones((128, 1), BF16_NP)
    onesrow = np.ones((1, 128), np.float32)
    for core in range(NCORES):
        a = a_all[core]
        w1blk = np.zeros((128, 512), BF16_NP)
        w2blk = np.zeros((128, 512), BF16_NP)
        for b in range(2):
            for ci in range(2):
                cc = b * 2 + ci
                w1blk[:, cc * 128 + b * 64: cc * 128 + b * 64 + 64] = \
                    w1[a[b], ci * 128:(ci + 1) * 128, :]
                w2blk[:, cc * 128 + b * 64: cc * 128 + b * 64 + 64] = \
                    w2[a[b], ci * 128:(ci + 1) * 128, :]
        vablk = np.zeros((128, 2), BF16_NP)
        ybias = np.zeros((128, 1), np.float32)
        for b in range(2):
            vablk[b * 64:(b + 1) * 64, b] = v[a[b]]
            ybias[b * 64:(b + 1) * 64, 0] = b1[a[b]] + b2[a[b]]
        m = {
            "memT": memT_all[core], "memr": memr_all[core],
            "w1blk": w1blk, "w2blk": w2blk,
            "vablk": vablk, "ybias": ybias,
            "wsru": wsru, "bsru": bsru,
            "onescol": onescol, "onesrow": onesrow,
        }
        if apply_mask:
            gb = [B2 * core + b for b in range(B2)]
            mk = np.empty((128, 4), np.float32)
            for lh in range(2):
                for b in range(2):
                    mk[:, lh * 2 + b] = np.where(
                        x_mask[gb[b], lh * 128:(lh + 1) * 128], 0.0, 1.0)
            m["maskmul"] = mk
        in_maps.append(m)
    return in_maps, apply_mask


def assemble_output(results):
    y = np.empty((B, S, D), np.float32)
    for core in range(NCORES):
        outT = results[core]["outT"].astype(np.float32)  # (2dh,128dp,512C)
        oc = outT.reshape(2, 128, 2, 2, 128)       # [dh, dp, ck, b, q]
        for b in range(B2):
            # y[b, s, dh*128+dp]; s = ck*128+q
            yb = oc[:, :, :, b, :]                 # (dh, dp, ck, q)
            yb = yb.transpose(2, 3, 0, 1).reshape(S, D)
            y[B2 * core + b] = yb
    return y


# ---- cached-jit SPMD runner (axon/PJRT path) --------------------------------
# run_bass_kernel_spmd re-traces and re-jits a fresh closure on every call,
# which costs ~1s of wall clock per invocation under the PJRT redirect. Build
# the sharded executable once per Bass module and reuse it.
_RUN_CACHE = {}


def _make_runner(nc):
    import jax
    from jax.experimental.shard_map import shard_map
    from jax.sharding import Mesh, PartitionSpec
    import concourse.mybir as _mybir
    from concourse import bass2jax as B2J

    B2J.install_neuronx_cc_hook()
    partition_name = (nc.partition_id_tensor.name
                      if nc.partition_id_tensor else None)
    in_names, out_names, out_avals, zero_outs = [], [], [], []
    for alloc in nc.m.functions[0].allocations:
        if not isinstance(alloc, _mybir.MemoryLocationSet):
            continue
        name = alloc.memorylocations[0].name
        if alloc.kind == "ExternalInput":
            if name != partition_name:
                in_names.append(name)
        elif alloc.kind == "ExternalOutput":
            shape = tuple(alloc.tensor_shape)
            dtype = _mybir.dt.np(alloc.dtype)
            out_names.append(name)
            out_avals.append(jax.core.ShapedArray(shape, dtype))
            zero_outs.append(np.zeros((NCORES * shape[0], *shape[1:]), dtype))
    n_params = len(in_names)
    all_names = in_names + out_names
    if partition_name is not None:
        all_names.append(partition_name)
    donate = tuple(range(n_params, n_params + len(out_names)))

    def _body(*args):
        operands = list(args)
        if partition_name is not None:
            operands.append(B2J.partition_id_tensor())
        return tuple(B2J._bass_exec_p.bind(
            *operands, out_avals=tuple(out_avals), in_names=tuple(all_names),
            out_names=tuple(out_names), lowering_input_output_aliases=(),
            sim_require_finite=True, sim_require_nnan=True, nc=nc))

    devices = jax.devices()[:NCORES]
    mesh = Mesh(np.asarray(devices), ("core",))
    nio = n_params + len(out_names)
    sharded = jax.jit(
        shard_map(_body, mesh=mesh, in_specs=(PartitionSpec("core"),) * nio,
                  out_specs=(PartitionSpec("core"),) * len(out_names),
                  check_rep=False),
        donate_argnums=donate, keep_unused=True)

    def run(in_maps):
        concat_in = [
            np.concatenate([np.asarray(in_maps[c][nm]) for c in range(NCORES)],
                           axis=0)
            for nm in in_names]
        out_arrs = sharded(*concat_in, *zero_outs)
        return [
            {nm: np.asarray(out_arrs[i]).reshape(NCORES, *out_avals[i].shape)[c]
             for i, nm in enumerate(out_names)}
            for c in range(NCORES)]

    return run


def _run_spmd(nc, in_maps):
    from concourse._compat import axon_active
    if not axon_active():
        return run_bass_kernel_spmd(nc, in_maps, list(range(NCORES))).results
    key = id(nc)
    if key not in _RUN_CACHE:
        _RUN_CACHE[key] = _make_runner(nc)
    return _RUN_CACHE[key](in_maps)


def kernel(**inputs) -> np.ndarray:
    in_maps, apply_mask = make_in_maps(**inputs)
    nc = _get_nc(apply_mask)
    results = _run_spmd(nc, in_maps)
    return assemble_output(results)


# revision 37
# speedup vs baseline: 1.0016x; 1.0016x over previous
"""MatchBRNN Trainium2 kernel: 2-layer action-conditioned-attention +
bidirectional SRU, data-parallel over batch on 8 NeuronCores (B=16 ->
2 batches/core).

Layout C: on-chip column index for (position q, batch b) is
    C(q, b) = (q // 128) * 256 + b * 128 + (q % 128)
i.e. 128-position chunks, batch-major inside a chunk.

Per-core structure (four (layer, chunk) "stages", software-pipelined):
  xtT[(b,k), l] = (x_b @ w1a_b)^T          (block-diag over b; layer-invariant)
  per stage: ytT = (out @ w2a)^T + bias; per 16-s block: tp = xt + yt_s on
  DVE (a few lead slices fused into ACT via tanh-bias to hide stage-start
  latency), one big ACT tanh per block, per-s PE score matmuls vs block-diag
  va; the softmax/pools/SRU tail of stage k-1 is emitted interleaved into
  stage k's blocks so neither ACT nor DVE stalls at chunk boundaries; the
  final stage's tail is split into two q-granules so most of it hides behind
  the last blocks.

All matmul operands staged in bf16 (memT/memr/w1/w2/wsru/eT/pools/h0);
gates/scan in f32; output DMA'd as bf16.  Measured on trn2: ~167 us device
makespan per core (NTFF), rel err ~5e-3 vs the fp32 jax reference.
"""
import numpy as np
import concourse.bass as bass
import concourse.mybir as mybir
import concourse.tile as tile
from concourse.bass_utils import run_bass_kernel_spmd

AF = mybir.ActivationFunctionType
OP = mybir.AluOpType
F32 = mybir.dt.float32
BF16 = mybir.dt.bfloat16
BF16_NP = mybir.dt.np(BF16)

B, S, D = 16, 256, 256
H, NL, A, K = 128, 2, 8, 64
NCORES = 8
B2 = B // NCORES
# fused (ACT-bias) tanh slices: only the first block of each stage
# fuses a few slices so ACT can start before the DVE tp-build catches up
FUSED_FIRST = 10
# final-tail granules: (q-offset, width); last one is the exposed tail
GRAN_BOUNDS = ((0, 64), (64, 64))


def _split_excess_waits(nc, max_waits=1):
    """walrus in this toolchain rejects >1 sem-wait per instruction; hoist
    extras onto same-engine NoOps inserted just before the instruction."""
    n = 0
    for f in nc.m.functions:
        for bb in f.blocks:
            out = []
            for inst in bb.instructions:
                si = inst.sync_info
                waits = list(si.on_wait) if si is not None and si.on_wait else []
                if len(waits) > max_waits:
                    keep, extra = waits[-max_waits:], waits[:-max_waits]
                    for w in extra:
                        n += 1
                        out.append(mybir.InstNoOp(
                            name=f"{inst.name}_ws{n}", engine=inst.engine,
                            ins=[], outs=[],
                            sync_info=mybir.SyncInfo(on_wait=[w], on_update=[])))
                    inst.sync_info = mybir.SyncInfo(
                        on_wait=keep, on_update=list(si.on_update or []))
                out.append(inst)
            bb.instructions = out
    return n


def _build(apply_mask: bool):
    nc = bass.Bass("TRN2")
    dram = nc.dram_tensor
    memT_d = dram("memT", [128, 1024], BF16, kind="ExternalInput")
    memr_d = dram("memr", [128, 1024], BF16, kind="ExternalInput")
    w1_d = dram("w1blk", [128, 512], BF16, kind="ExternalInput")
    w2_d = dram("w2blk", [128, 512], BF16, kind="ExternalInput")
    va_d = dram("vablk", [128, 2], BF16, kind="ExternalInput")
    yb_d = dram("ybias", [128, 1], F32, kind="ExternalInput")
    ws_d = dram("wsru", [128, 8192], BF16, kind="ExternalInput")
    bs_d = dram("bsru", [128, 8], F32, kind="ExternalInput")
    oc_d = dram("onescol", [128, 1], BF16, kind="ExternalInput")
    or_d = dram("onesrow", [1, 128], F32, kind="ExternalInput")
    if apply_mask:
        mk_d = dram("maskmul", [128, 4], F32, kind="ExternalInput")
    outT_d = dram("outT", [2, 128, 512], BF16, kind="ExternalOutput")

    with tile.TileContext(nc) as tc:
        with (
            nc.allow_low_precision(reason="bf16 staging is intentional"),
            tc.tile_pool(name="const", bufs=1) as cp,
            tc.tile_pool(name="work", bufs=1) as wp,
            tc.tile_pool(name="blk", bufs=3) as bp,
            tc.tile_pool(name="ps", bufs=1, space="PSUM") as ps,
        ):
            # ACT table preload: tiny tanh right at t=0, concurrent with DMAs
            warm = cp.tile([128, 1], F32, tag="warm")
            nc.vector.memset(warm[:], 0.0)
            nc.scalar.activation(warm[:], warm[:], AF.Tanh)

            memT = cp.tile([128, 1024], BF16, tag="memT")
            memr = cp.tile([128, 1024], BF16, tag="memr")
            w1 = cp.tile([128, 512], BF16, tag="w1")
            w2 = cp.tile([128, 512], BF16, tag="w2")
            va = cp.tile([128, 2], BF16, tag="va")
            yb = cp.tile([128, 1], F32, tag="yb")
            wsru = cp.tile([128, 8192], BF16, tag="wsru")
            bsru = cp.tile([128, 8], F32, tag="bsru")
            onc = cp.tile([128, 1], BF16, tag="onc")
            onr = cp.tile([1, 128], F32, tag="onr")
            # priority loads first (feed the first chunk)
            for q in (0, 2, 1, 3):
                nc.sync.dma_start(memT[:, q * 256:(q + 1) * 256],
                                  memT_d[:, q * 256:(q + 1) * 256])
            for t, d in ((w1, w1_d), (w2, w2_d), (va, va_d), (yb, yb_d),
                         (onc, oc_d), (onr, or_d), (bsru, bs_d)):
                nc.sync.dma_start(t[:], d[:])
            nc.sync.dma_start(memr[:], memr_d[:])
            for q in range(4):
                nc.sync.dma_start(wsru[:, q * 2048:(q + 1) * 2048],
                                  ws_d[:, q * 2048:(q + 1) * 2048])
            if apply_mask:
                mk = cp.tile([128, 4], F32, tag="mk")
                nc.sync.dma_start(mk[:], mk_d[:])

            h0 = [wp.tile([128, 512], BF16, tag=f"h0{d}", name=f"h0{d}")
                  for d in range(2)]
            h1 = [wp.tile([128, 512], BF16, tag=f"h1{d}", name=f"h1{d}")
                  for d in range(2)]

            # PSUM: 8 banks, all as (128, 512) f32 tiles
            u_ps = {}
            for jj in range(4):
                u_ps[jj] = ps.tile([128, 512], F32, tag=f"u{jj}", name=f"ups{jj}")
            sc_ps = [ps.tile([128, 512], F32, tag=f"sc{h}", name=f"scps{h}")
                     for h in range(2)]
            pn_ps = [ps.tile([128, 512], F32, tag=f"pn{dh}", name=f"pnps{dh}")
                     for dh in range(2)]

            # ---- layer-invariant xtT: contract (b, d-half), block-diag w1 ----
            xt16 = wp.tile([128, 256], BF16, tag="xt16")

            def emit_xtT(ck):
                co = ck * 256
                for cc in range(4):
                    b, ci = cc // 2, cc % 2
                    nc.tensor.matmul(
                        sc_ps[0][:, co:co + 128], w1[:, cc * 128:(cc + 1) * 128],
                        memT[:, ci * 512 + co + b * 128:
                             ci * 512 + co + (b + 1) * 128],
                        start=(cc == 0), stop=(cc == 3))
                nc.vector.tensor_copy(xt16[:, ck * 128:(ck + 1) * 128],
                                      sc_ps[0][:, co:co + 128])

            emit_xtT(0)

            # per-layer tiles (tag-shared across layers; tails never overlap)
            yts = [wp.tile([128, 256], F32, tag=f"yt{li}", name=f"yt{li}")
                   for li in range(NL)]
            eTs = [wp.tile([128, 1024], BF16, tag=f"eT{li}", name=f"eT{li}")
                   for li in range(NL)]
            rzs = [wp.tile([1, 512], F32, tag=f"rz{li}", name=f"rz{li}")
                   for li in range(NL)]
            rzbs = [wp.tile([128, 512], F32, tag=f"rzb{li}", name=f"rzb{li}")
                    for li in range(NL)]
            poolsTs = [[wp.tile([128, 512], BF16, tag=f"pT{li}{dh}",
                                name=f"poolsT{li}_{dh}") for dh in range(2)]
                       for li in range(NL)]
            gts = [[{nm: wp.tile([128, 512], F32, tag=f"{nm}{li}{dr}",
                                 name=f"{nm}_{li}_{dr}")
                     for nm in ("tf", "f", "g", "bin", "c", "tc2", "tr",
                                "dd", "rd2")} for dr in range(2)]
                   for li in range(NL)]

            def emit_ytT(li, ck):
                co = ck * 256
                for cc in range(4):
                    b, ci = cc // 2, cc % 2
                    if li == 0:
                        rhs = memT[:, ci * 512 + co + b * 128:
                                   ci * 512 + co + (b + 1) * 128]
                    else:
                        rhs = h0[ci][:, co + b * 128: co + (b + 1) * 128]
                    nc.tensor.matmul(
                        sc_ps[1][:, co:co + 128],
                        w2[:, cc * 128:(cc + 1) * 128], rhs,
                        start=(cc == 0), stop=(cc == 3))
                nc.vector.tensor_scalar(
                    yts[li][:, ck * 128:(ck + 1) * 128],
                    sc_ps[1][:, co:co + 128], yb[:], None, OP.add)

            def emit_tp(li, ck, blk, nf):
                """DVE tp-build for slices j >= nf of one 16-s block."""
                yt = yts[li]
                tp = bp.tile([128, 4096], BF16, tag="tpre")
                for j in range(nf, 16):
                    s = ck * 128 + blk * 16 + j
                    nc.vector.tensor_scalar(
                        tp[:, (j - nf) * 256:(j - nf + 1) * 256], xt16[:],
                        yt[:, s:s + 1], None, OP.add)
                return tp

            def emit_block(li, ck, blk, nf=0, tp=None):
                """tanh + scores for one 16-s block (tp may be prebuilt)."""
                co = ck * 256
                yt = yts[li]
                tb = bp.tile([128, 4096], BF16, tag="tblk")
                if tp is None:
                    tp = emit_tp(li, ck, blk, nf)
                # fused ACT-bias tanh for j < nf (no DVE work)
                for j in range(nf):
                    s = ck * 128 + blk * 16 + j
                    nc.scalar.activation(tb[:, j * 256:(j + 1) * 256],
                                         xt16[:], AF.Tanh,
                                         bias=yt[:, s:s + 1])
                nw = (16 - nf) * 256
                nc.scalar.activation(tb[:, nf * 256: nf * 256 + nw],
                                     tp[:, 0:nw], AF.Tanh)
                for j in range(16):
                    q = blk * 16 + j
                    for h in range(2):
                        nc.tensor.matmul(
                            sc_ps[h][:, co + q: co + q + 129: 128],
                            tb[:, j * 256 + h * 128: j * 256 + (h + 1) * 128],
                            va[:], start=True, stop=True)

            def emit_tail_piece(li, ck, piece, g=None):
                """Tail ops for one chunk; g=None does the full 128-q chunk,
                g in (0,1) does the half-granule q in [g*64, g*64+64)."""
                co = ck * 256
                eT, rz, rzb = eTs[li], rzs[li], rzbs[li]
                poolsT = poolsTs[li]
                if g is None:
                    spans = [(co, 256)]           # contiguous, both batches
                    bspans = [(co, 128), (co + 128, 128)]
                else:
                    g0, gw = GRAN_BOUNDS[g]
                    spans = [(co + g0, gw), (co + 128 + g0, gw)]
                    bspans = spans
                if piece == 0:
                    for h in range(2):
                        for st, ln in spans:
                            nc.scalar.activation(
                                eT[:, h * 512 + st: h * 512 + st + ln],
                                sc_ps[h][:, st:st + ln], AF.Exp)
                    if apply_mask:
                        for h in range(2):
                            for st, ln in bspans:
                                b = (st - co) // 128
                                sl = eT[:, h * 512 + st: h * 512 + st + ln]
                                nc.vector.tensor_scalar(
                                    sl, sl, mk[:, h * 2 + b: h * 2 + b + 1],
                                    None, OP.mult)
                elif piece == 1:
                    for st, ln in spans:
                        for h in range(2):
                            nc.tensor.matmul(
                                pn_ps[0][0:1, st:st + ln], onc[:],
                                eT[:, h * 512 + st: h * 512 + st + ln],
                                start=(h == 0), stop=(h == 1))
                        nc.vector.reciprocal(rz[0:1, st:st + ln],
                                             pn_ps[0][0:1, st:st + ln])
                elif piece == 2:
                    for st, ln in bspans:
                        nc.tensor.matmul(
                            pn_ps[1][:, st:st + ln], onr[:],
                            rz[0:1, st:st + ln], start=True, stop=True)
                    for st, ln in spans:
                        nc.vector.tensor_copy(rzb[:, st:st + ln],
                                              pn_ps[1][:, st:st + ln])
                elif piece == 3:
                    for dh in range(2):
                        for st, ln in bspans:
                            b = (st - co) // 128
                            for lh in range(2):
                                nc.tensor.matmul(
                                    pn_ps[dh][:, st:st + ln],
                                    memr[:, lh * 512 + b * 256 + dh * 128:
                                         lh * 512 + b * 256 + (dh + 1) * 128],
                                    eT[:, lh * 512 + st: lh * 512 + st + ln],
                                    start=(lh == 0), stop=(lh == 1))
                        for st, ln in spans:
                            nc.vector.scalar_tensor_tensor(
                                poolsT[dh][:, st:st + ln],
                                pn_ps[dh][:, st:st + ln],
                                1.0, rzb[:, st:st + ln], OP.mult, OP.mult)
                elif piece in (4, 6):
                    dr = 0 if piece == 4 else 1
                    for st, ln in spans:
                        for c in range(4):
                            if c < 2:
                                rhs = (memT[:, c * 512 + st: c * 512 + st + ln]
                                       if li == 0 else h0[c][:, st:st + ln])
                            else:
                                rhs = poolsT[c - 2][:, st:st + ln]
                            for jj in range(4):
                                w_off = (((li * 2 + dr) * 16) + c * 4 + jj) * 128
                                nc.tensor.matmul(
                                    u_ps[jj][:, st:st + ln],
                                    wsru[:, w_off:w_off + 128], rhs,
                                    start=(c == 0), stop=(c == 3))
                elif piece in (5, 7):
                    dr = 0 if piece == 5 else 1
                    bcol = (li * 2 + dr) * 2
                    gt = gts[li][dr]
                    tf_, f_, g_, bin_, c_, tc2, tr_, dd_, rd2_ = (
                        gt["tf"], gt["f"], gt["g"], gt["bin"], gt["c"],
                        gt["tc2"], gt["tr"], gt["dd"], gt["rd2"])
                    for st, ln in spans:
                        nc.scalar.activation(tf_[:, st:st + ln],
                                             u_ps[1][:, st:st + ln], AF.Tanh,
                                             bias=bsru[:, bcol:bcol + 1],
                                             scale=0.5)
                        nc.vector.tensor_scalar(f_[:, st:st + ln],
                                                tf_[:, st:st + ln], 0.5, 0.5,
                                                OP.mult, OP.add)
                        nc.vector.tensor_scalar(g_[:, st:st + ln],
                                                tf_[:, st:st + ln], -0.5, 0.5,
                                                OP.mult, OP.add)
                        nc.vector.tensor_tensor(bin_[:, st:st + ln],
                                                g_[:, st:st + ln],
                                                u_ps[0][:, st:st + ln], OP.mult)
                    for st, ln in bspans:
                        qoff = (st - co) % 128
                        nsc = ln
                        if qoff == 0 and ck == 0:
                            init = 0.0
                        elif qoff == 0:
                            init = c_[:, st - 129: st - 128]
                        else:
                            init = c_[:, st - 1: st]
                        nc.vector.tensor_tensor_scan(
                            c_[:, st:st + nsc], f_[:, st:st + nsc],
                            bin_[:, st:st + nsc], init, OP.mult, OP.add)
                    for st, ln in spans:
                        nc.scalar.activation(tc2[:, st:st + ln],
                                             c_[:, st:st + ln], AF.Tanh)
                        nc.scalar.activation(tr_[:, st:st + ln],
                                             u_ps[2][:, st:st + ln], AF.Tanh,
                                             bias=bsru[:, bcol + 1:bcol + 2],
                                             scale=0.5)
                        nc.vector.tensor_tensor(dd_[:, st:st + ln],
                                                tc2[:, st:st + ln],
                                                u_ps[3][:, st:st + ln],
                                                OP.subtract)
                        nc.vector.scalar_tensor_tensor(
                            rd2_[:, st:st + ln], tr_[:, st:st + ln], 1.0,
                            dd_[:, st:st + ln], OP.add, OP.mult)
                        h_t = h0[dr] if li == 0 else h1[dr]
                        nc.vector.scalar_tensor_tensor(
                            h_t[:, st:st + ln], rd2_[:, st:st + ln], 0.5,
                            u_ps[3][:, st:st + ln], OP.mult, OP.add)
                        if li == 1 and piece == 7:
                            for dh in range(2):
                                nc.sync.dma_start(outT_d[dh, :, st:st + ln],
                                                  h1[dh][:, st:st + ln])

            # ---- pipelined emission over 4 stages (li, ck) ----
            stages = [(0, 0), (0, 1), (1, 0), (1, 1)]
            # tail pieces of stage k-1 interleaved into stage k's blocks;
            # ytT of stage k+1 hoisted after block YTT_AT so the PE reaches
            # it before the last blocks' score matmuls
            TAIL_SCHED = {0: [0, 1], 1: [2, 3], 2: [4], 3: [5], 4: [6], 5: [7]}
            YTT_AT = 6
            # final-stage tail: split into NGRAN q-granules; granule g needs
            # scores only from blocks < 2(g+1), so all but the last granule
            # hide behind the remaining blocks
            # ordering constraints: piece p of granule g must follow piece p
            # of the stage-2 tail (TAIL_SCHED: p5 at blk3, p7 at blk5) and of
            # granule g-1 (scan chaining), and granule g's piece 0 needs the
            # scores of blocks < 2*(g+1)
            G_SCHED = {4: [(0, 0), (0, 1)],
                       5: [(0, 2), (0, 3)],
                       6: [(0, 4), (0, 5)],
                       7: [(0, 6), (0, 7)]}
            G_END = [(1, p) for p in range(8)]
            emit_ytT(0, 0)
            emit_xtT(1)
            for k, (li, ck) in enumerate(stages):
                for blk in range(8):
                    emit_block(li, ck, blk,
                               nf=FUSED_FIRST if blk == 0 else 0)
                    if k >= 1:
                        pli, pck = stages[k - 1]
                        for piece in TAIL_SCHED.get(blk, []):
                            emit_tail_piece(pli, pck, piece)
                    if k == 3:
                        for gg, piece in G_SCHED.get(blk, []):
                            emit_tail_piece(1, 1, piece, g=gg)
                    if blk == YTT_AT and k + 1 < 4:
                        emit_ytT(*stages[k + 1])
            for gg, piece in G_END:
                emit_tail_piece(1, 1, piece, g=gg)

    _split_excess_waits(nc)
    return nc


_CACHE = {}


def _get_nc(apply_mask: bool):
    if apply_mask not in _CACHE:
        _CACHE[apply_mask] = _build(apply_mask)
    return _CACHE[apply_mask]


def _c_layout_unused(arr_pos_b):
    """(pos, b, ...) -> columns in layout C: [ck*256 + b*128 + q]."""
    P2, Bb = arr_pos_b.shape[0], arr_pos_b.shape[1]
    rest = arr_pos_b.shape[2:]
    a = arr_pos_b.reshape(2, 128, Bb, *rest)       # (ck, q, b, ...)
    a = np.moveaxis(a, 2, 1)                       # (ck, b, q, ...)
    return a.reshape(512, *rest)


def make_in_maps(x, x_mask, actions, w1, b1, w2, b2, v,
                 sru_w_f, sru_b_f, sru_w_b, sru_b_b):
    x = np.asarray(x, np.float32)
    x_mask = np.asarray(x_mask)
    actions = np.asarray(actions).astype(np.int64)
    w1 = np.asarray(w1, np.float32); b1 = np.asarray(b1, np.float32)
    w2 = np.asarray(w2, np.float32); b2 = np.asarray(b2, np.float32)
    v = np.asarray(v, np.float32)

    apply_mask = bool(x_mask.any())

    # wsru[:, ((li*2+dr)*16 + c*4 + jj)*128 + m] = sru_w[dr][li, c*128+dp, jj*128+m]
    sw = np.stack([np.asarray(sru_w_f, np.float32),
                   np.asarray(sru_w_b, np.float32)], 1)   # (li, dr, 512, 512)
    blk = sw.reshape(NL, 2, 4, 128, 4, 128)               # li dr c dp jj m
    wsru = np.ascontiguousarray(
        blk.transpose(3, 0, 1, 2, 4, 5).reshape(128, 8192)).astype(BF16_NP)
    sb = np.stack([np.asarray(sru_b_f, np.float32),
                   np.asarray(sru_b_b, np.float32)], 1)   # (li, dr, 256)
    bsru = np.ascontiguousarray(
        (0.5 * sb.reshape(NL, 2, 2, 128)).transpose(3, 0, 1, 2).reshape(128, 8))

    # layout C over all cores at once
    xs = x.reshape(NCORES, B2, S, D)
    arr = xs.transpose(0, 2, 1, 3)                         # (core, l, b, d)
    colsC = (arr.reshape(NCORES, 2, 128, B2, D)
             .transpose(0, 1, 3, 2, 4).reshape(NCORES, 512, D))
    # memT[dp, dh*512 + C] = colsC[C, dh*128+dp]
    tmp = colsC.reshape(NCORES, 512, 2, 128)               # (core, C, dh, dp)
    memT_all = np.ascontiguousarray(
        tmp.transpose(0, 3, 2, 1)                          # (core, dp, dh, C)
        .reshape(NCORES, 128, 1024)).astype(BF16_NP)
    # memr[lp, lh*512 + b*256 + d] = x[b, lh*128+lp, d]
    memr_all = np.ascontiguousarray(
        arr.reshape(NCORES, 2, 128, B2 * D).transpose(0, 2, 1, 3)
        .reshape(NCORES, 128, 1024)).astype(BF16_NP)

    a_all = actions.reshape(NCORES, B2)
    in_maps = []
    onescol = np.ones((128, 1), BF16_NP)
    onesrow = np.ones((1, 128), np.float32)
    for core in range(NCORES):
        a = a_all[core]
        w1blk = np.zeros((128, 512), BF16_NP)
        w2blk = np.zeros((128, 512), BF16_NP)
        for b in range(2):
            for ci in range(2):
                cc = b * 2 + ci
                w1blk[:, cc * 128 + b * 64: cc * 128 + b * 64 + 64] = \
                    w1[a[b], ci * 128:(ci + 1) * 128, :]
                w2blk[:, cc * 128 + b * 64: cc * 128 + b * 64 + 64] = \
                    w2[a[b], ci * 128:(ci + 1) * 128, :]
        vablk = np.zeros((128, 2), BF16_NP)
        ybias = np.zeros((128, 1), np.float32)
        for b in range(2):
            vablk[b * 64:(b + 1) * 64, b] = v[a[b]]
            ybias[b * 64:(b + 1) * 64, 0] = b1[a[b]] + b2[a[b]]
        m = {
            "memT": memT_all[core], "memr": memr_all[core],
            "w1blk": w1blk, "w2blk": w2blk,
            "vablk": vablk, "ybias": ybias,
            "wsru": wsru, "bsru": bsru,
            "onescol": onescol, "onesrow": onesrow,
        }
        if apply_mask:
            gb = [B2 * core + b for b in range(B2)]
            mk = np.empty((128, 4), np.float32)
            for lh in range(2):
                for b in range(2):
                    mk[:, lh * 2 + b] = np.where(
                        x_mask[gb[b], lh * 128:(lh + 1) * 128], 0.0, 1.0)
            m["maskmul"] = mk
        in_maps.append(m)
    return in_maps, apply_mask


def assemble_output(results):
    y = np.empty((B, S, D), np.float32)
    for core in range(NCORES):
        outT = results[core]["outT"].astype(np.float32)  # (2dh,128dp,512C)
        oc = outT.reshape(2, 128, 2, 2, 128)       # [dh, dp, ck, b, q]
        for b in range(B2):
            # y[b, s, dh*128+dp]; s = ck*128+q
            yb = oc[:, :, :, b, :]                 # (dh, dp, ck, q)
            yb = yb.transpose(2, 3, 0, 1).reshape(S, D)
            y[B2 * core + b] = yb
    return y


# ---- cached-jit SPMD runner (axon/PJRT path) --------------------------------
# run_bass_kernel_spmd re-traces and re-jits a fresh closure on every call,
# which costs ~1s of wall clock per invocation under the PJRT redirect. Build
# the sharded executable once per Bass module and reuse it.
_RUN_CACHE = {}


def _make_runner(nc):
    import jax
    from jax.experimental.shard_map import shard_map
    from jax.sharding import Mesh, PartitionSpec
    import concourse.mybir as _mybir
    from concourse import bass2jax as B2J

    B2J.install_neuronx_cc_hook()
    partition_name = (nc.partition_id_tensor.name
                      if nc.partition_id_tensor else None)
    in_names, out_names, out_avals, zero_outs = [], [], [], []
    for alloc in nc.m.functions[0].allocations:
        if not isinstance(alloc, _mybir.MemoryLocationSet):
            continue
        name = alloc.memorylocations[0].name
        if alloc.kind == "ExternalInput":
            if name != partition_name:
                in_names.append(name)
        elif alloc.kind == "ExternalOutput":
            shape = tuple(alloc.tensor_shape)
            dtype = _mybir.dt.np(alloc.dtype)
            out_names.append(name)
            out_avals.append(jax.core.ShapedArray(shape, dtype))
            zero_outs.append(np.zeros((NCORES * shape[0], *shape[1:]), dtype))
    n_params = len(in_names)
    all_names = in_names + out_names
    if partition_name is not None:
        all_names.append(partition_name)
    donate = tuple(range(n_params, n_params + len(out_names)))

    def _body(*args):
        operands = list(args)
        if partition_name is not None:
            operands.append(B2J.partition_id_tensor())
        return tuple(B2J._bass_exec_p.bind(
            *operands, out_avals=tuple(out_avals), in_names=tuple(all_names),
            out_names=tuple(out_names), lowering_input_output_aliases=(),
            sim_require_finite=True, sim_require_nnan=True, nc=nc))

    devices = jax.devices()[:NCORES]
    mesh = Mesh(np.asarray(devices), ("core",))
    nio = n_params + len(out_names)
    sharded = jax.jit(
        shard_map(_body, mesh=mesh, in_specs=(PartitionSpec("core"),) * nio,
                  out_specs=(PartitionSpec("core"),) * len(out_names),
                  check_rep=False),
        donate_argnums=donate, keep_unused=True)

    def run(in_maps):
        concat_in = [
            np.concatenate([np.asarray(in_maps[c][nm]) for c in range(NCORES)],
                           axis=0)
            for nm in in_names]
        out_arrs = sharded(*concat_in, *zero_outs)
        return [
            {nm: np.asarray(out_arrs[i]).reshape(NCORES, *out_avals[i].shape)[c]
             for i, nm in enumerate(out_names)}
            for c in range(NCORES)]

    return run


def _run_spmd(nc, in_maps):
    from concourse._compat import axon_active
    if not axon_active():
        return run_bass_kernel_spmd(nc, in_maps, list(range(NCORES))).results
    key = id(nc)
    if key not in _RUN_CACHE:
        _RUN_CACHE[key] = _make_runner(nc)
    return _RUN_CACHE[key](in_maps)


def kernel(**inputs) -> np.ndarray:
    in_maps, apply_mask = make_in_maps(**inputs)
    nc = _get_nc(apply_mask)
    results = _run_spmd(nc, in_maps)
    return assemble_output(results)
